# revision 9
# baseline (speedup 1.0000x reference)
"""CapsulePooling2D kernel for Trainium2, 8-core batch-data-parallel.

Full input x: (32, 64, 64, 256) fp32 -> output (32, 32, 32, 256) fp32.
Each NeuronCore handles 4 samples.

Math (per independent 2x2 spatial block of 4 pixels p0..p3, 256 channels):
  G[a,b] = x_a . x_b                      (4x4 Gram, from fp16-rounded x)
  step1: d1 = G @ 1 ; sigma1 = sum(G);  s1 = d1 / (4*(1+EPS) + sqrt(sigma1))
  step2: e2 = exp((s1 - max s1)/2); u = G @ e2
         s2 = u / ((sum e2 + 4EPS)*(1+EPS) + sqrt(e2^T G e2)); score = s1+s2
  step3: e3 = exp((score - max score)/2); w = e3 / (sum e3 + 4EPS)
         out = sum_k w_k * x_k
(The reference's step-3 score update is dead code for the output.)

Layout: tile = 128 blocks on partitions (4 block-rows x 32 block-cols),
free dim = (pixel 4, channel 256); x stored fp16 (SWDGE cast during DMA).
Off-diagonal Gram: fused scalar_tensor_tensor + accum on VectorE.
Diagonal Gram: Square + accum on ScalarE. Per-block scalar algebra batched
across all 32 tiles (sqrt via exp(0.5*ln) keeps one ACT table set).
Weighted output sum: V tensor_scalar + chained scalar_tensor_tensor, with
the last pixel's multiply on ScalarE (scaled Copy). GpSimd does only DMA
descriptor generation (its compute contends with VectorE's SBUF port).
"""

import sys

if "/opt/trn_rl_repo" not in sys.path:
    sys.path.insert(0, "/opt/trn_rl_repo")

import numpy as np

N_CORES = 8
B_FULL, H, W, C = 32, 64, 64, 256
B = B_FULL // N_CORES          # 4 samples per core
HO, WO = H // 2, W // 2
ROWS_PER_TILE = 8              # image rows per tile -> 4 block-rows x 32 = 128 blocks
TILES_PER_SAMPLE = H // ROWS_PER_TILE  # 8
NT = B * TILES_PER_SAMPLE      # 32 tiles per core
EPS = 1e-7

_cache = {}


def _build_nc(act_chain=True):
    import concourse.bacc as bacc
    import concourse.tile as tile
    import concourse.mybir as mybir

    fp32 = mybir.dt.float32
    fp16 = mybir.dt.float16
    F = mybir.ActivationFunctionType
    OP = mybir.AluOpType
    AX = mybir.AxisListType

    nc = bacc.Bacc("TRN2", num_devices=N_CORES)
    x = nc.dram_tensor("x", [B, H, W, C], fp32, kind="ExternalInput").ap()
    out = nc.dram_tensor("out", [B, HO, WO, C], fp32, kind="ExternalOutput").ap()

    with tile.TileContext(nc) as tc:
        with (
            tc.tile_pool(name="xp", bufs=1) as xp,
            tc.tile_pool(name="gp", bufs=1) as gp,
            tc.tile_pool(name="sp", bufs=1) as sp,
            tc.tile_pool(name="scr", bufs=2) as scrp,
            tc.tile_pool(name="op", bufs=4) as outp,
        ):
            # ---- all of x, fp16, one buffer: [128, NT, 4, 256] ----
            X = xp.tile([128, NT * 4 * C], fp16, tag="X", name="X")
            Xv = X[:].rearrange("p (t k c) -> p t k c", t=NT, k=4)
            for t in range(NT):
                s, ti = divmod(t, TILES_PER_SAMPLE)
                src = x[s, ROWS_PER_TILE * ti : ROWS_PER_TILE * (ti + 1), :, :]
                # [ii, di, j, (dj c)]
                src = src.rearrange("(ii di) (j dj) c -> ii di j (dj c)", di=2, dj=2)
                for di in range(2):
                    nc.gpsimd.dma_start(
                        out=Xv[:, t, 2 * di : 2 * di + 2, :],
                        in_=src[:, di, :, :],
                    )

            G_all = gp.tile([128, NT * 16], fp32, tag="G", name="G_all")
            Gv = G_all[:].rearrange("p (t a b) -> p t a b", t=NT, a=4)

            # ---- Gram: diagonals on ScalarE, off-diagonals on VectorE ----
            PAIRS = [(a, b) for a in range(4) for b in range(a, 4)]
            for t in range(NT):
                scrV = scrp.tile([128, C], fp16, tag="scrV", name="scrV")
                scrA = scrp.tile([128, C], fp16, tag="scrA", name="scrA")
                for (a, b) in PAIRS:
                    g_slot = G_all[:, t * 16 + a * 4 + b : t * 16 + a * 4 + b + 1]
                    if a == b:
                        nc.scalar.activation(
                            scrA[:], Xv[:, t, a, :], F.Square, accum_out=g_slot
                        )
                    else:
                        nc.vector.scalar_tensor_tensor(
                            out=scrV[:],
                            in0=Xv[:, t, a, :],
                            scalar=1.0,
                            in1=Xv[:, t, b, :],
                            op0=OP.bypass,
                            op1=OP.mult,
                            accum_out=g_slot,
                        )

            # ---- scalar algebra, batched across all NT tiles (fp32) ----
            # mirror upper triangle -> lower
            nc.vector.tensor_copy(Gv[:, :, 1:4, 0], Gv[:, :, 0, 1:4])
            nc.vector.tensor_copy(Gv[:, :, 2:4, 1], Gv[:, :, 1, 2:4])
            nc.vector.tensor_copy(Gv[:, :, 3, 2:3], Gv[:, :, 2, 3:4])

            def t4(name):
                return sp.tile([128, NT * 4], fp32, tag=name, name=name)

            def t1(name):
                return sp.tile([128, NT], fp32, tag=name, name=name)

            def sqrt_ln_exp(dst, src, tmp):
                # dst = sqrt(src) via exp(0.5*ln(src)) : one ACT table set
                nc.scalar.activation(tmp[:], src[:], F.Ln)
                nc.scalar.activation(dst[:], tmp[:], F.Exp, scale=0.5)

            d1 = t4("d1")
            d1v = d1[:].rearrange("p (t k) -> p t k", t=NT)
            nc.vector.tensor_reduce(d1v, Gv, axis=AX.X, op=OP.add)
            sig1 = t1("sig1")
            nc.vector.tensor_reduce(sig1[:], d1v, axis=AX.X, op=OP.add)
            sA = t1("sA")
            tmp1 = t1("tmp1")
            sqrt_ln_exp(sA, sig1, tmp1)
            den1 = t1("den1")
            nc.vector.tensor_scalar_add(den1[:], sA[:], 4.0 * (1.0 + EPS))
            r1 = t1("r1")
            nc.vector.reciprocal(r1[:], den1[:])
            score = t4("score")
            scv = score[:].rearrange("p (t k) -> p t k", t=NT)
            nc.vector.tensor_tensor(
                out=scv,
                in0=d1v,
                in1=r1[:].unsqueeze(2).broadcast_to([128, NT, 4]),
                op=OP.mult,
            )

            def softmax_weights(sc_v, ename):
                # e = exp((sc - max sc)/2) ; returns (e view, sum_e tile)
                m = t1("m" + ename)
                nc.vector.tensor_reduce(m[:], sc_v, axis=AX.X, op=OP.max)
                sh = t4("sh" + ename)
                shv = sh[:].rearrange("p (t k) -> p t k", t=NT)
                nc.vector.tensor_tensor(
                    out=shv,
                    in0=sc_v,
                    in1=m[:].unsqueeze(2).broadcast_to([128, NT, 4]),
                    op=OP.subtract,
                )
                e = t4("e" + ename)
                ev = e[:].rearrange("p (t k) -> p t k", t=NT)
                nc.scalar.activation(e[:], sh[:], F.Exp, scale=0.5)
                se = t1("se" + ename)
                nc.vector.tensor_reduce(se[:], ev, axis=AX.X, op=OP.add)
                return ev, se

            e2v, se2 = softmax_weights(scv, "2")
            # u = G @ e2  (per block)
            P4 = sp.tile([128, NT * 16], fp32, tag="P4", name="P4")
            P4v = P4[:].rearrange("p (t a b) -> p t a b", t=NT, a=4)
            nc.vector.tensor_tensor(
                out=P4v,
                in0=Gv,
                in1=e2v.unsqueeze(2).broadcast_to([128, NT, 4, 4]),
                op=OP.mult,
            )
            u = t4("u")
            uv = u[:].rearrange("p (t k) -> p t k", t=NT)
            nc.vector.tensor_reduce(uv, P4v, axis=AX.X, op=OP.add)
            # btb = e2 . u
            eu = t4("eu")
            euv = eu[:].rearrange("p (t k) -> p t k", t=NT)
            nc.vector.tensor_tensor(out=euv, in0=e2v, in1=uv, op=OP.mult)
            btb = t1("btb")
            nc.vector.tensor_reduce(btb[:], euv, axis=AX.X, op=OP.add)
            sB = t1("sB")
            tmp2 = t1("tmp2")
            sqrt_ln_exp(sB, btb, tmp2)
            DEN = t1("DEN")
            nc.vector.tensor_scalar(
                out=DEN[:],
                in0=se2[:],
                scalar1=4.0 * EPS,
                scalar2=1.0 + EPS,
                op0=OP.add,
                op1=OP.mult,
            )
            den2 = t1("den2")
            nc.vector.tensor_tensor(out=den2[:], in0=DEN[:], in1=sB[:], op=OP.add)
            r2 = t1("r2")
            nc.vector.reciprocal(r2[:], den2[:])
            # score2 = score + u*r2
            s2t = t4("s2t")
            s2tv = s2t[:].rearrange("p (t k) -> p t k", t=NT)
            nc.vector.tensor_tensor(
                out=s2tv,
                in0=uv,
                in1=r2[:].unsqueeze(2).broadcast_to([128, NT, 4]),
                op=OP.mult,
            )
            score2 = t4("score2")
            sc2v = score2[:].rearrange("p (t k) -> p t k", t=NT)
            nc.vector.tensor_tensor(out=sc2v, in0=scv, in1=s2tv, op=OP.add)

            e3v, se3 = softmax_weights(sc2v, "3")
            den3 = t1("den3")
            nc.vector.tensor_scalar_add(den3[:], se3[:], 4.0 * EPS)
            q3 = t1("q3")
            nc.vector.reciprocal(q3[:], den3[:])
            W3 = t4("W3")
            W3v = W3[:].rearrange("p (t k) -> p t k", t=NT)
            nc.vector.tensor_tensor(
                out=W3v,
                in0=e3v,
                in1=q3[:].unsqueeze(2).broadcast_to([128, NT, 4]),
                op=OP.mult,
            )

            # ---- weighted output sum + DMA out ----
            # o = w0*x0 ; o += w1*x1 ; o += w2*x2 (V chain)
            # m3 = w3*x3 on ScalarE (scaled Copy) ; final add on V
            W3f = W3[:]
            for t in range(NT):
                s, ti = divmod(t, TILES_PER_SAMPLE)

                def wsl(k):
                    return W3f[:, t * 4 + k : t * 4 + k + 1]

                o1 = outp.tile([128, C], fp16, tag="o1", name="o1")
                o2 = outp.tile([128, C], fp16, tag="o2", name="o2")
                m3 = outp.tile([128, C], fp16, tag="m3", name="m3")
                oT = outp.tile([128, C], fp16, tag="oT", name="oT")

                # m3 = w3*x3 on ScalarE seeds the V chain of fused mult-adds
                nc.scalar.activation(m3[:], Xv[:, t, 3, :], F.Copy, scale=wsl(3))
                nc.vector.scalar_tensor_tensor(
                    out=o1[:], in0=Xv[:, t, 0, :], scalar=wsl(0), in1=m3[:],
                    op0=OP.mult, op1=OP.add,
                )
                nc.vector.scalar_tensor_tensor(
                    out=o2[:], in0=Xv[:, t, 1, :], scalar=wsl(1), in1=o1[:],
                    op0=OP.mult, op1=OP.add,
                )
                nc.vector.scalar_tensor_tensor(
                    out=oT[:], in0=Xv[:, t, 2, :], scalar=wsl(2), in1=o2[:],
                    op0=OP.mult, op1=OP.add,
                )

                dst = out[s, 4 * ti : 4 * ti + 4, :, :].rearrange(
                    "ii j c -> (ii j) c"
                )
                nc.gpsimd.dma_start(out=dst, in_=oT[:])

    nc.compile()
    return nc


def _get_nc():
    if "nc" not in _cache:
        _cache["nc"] = _build_nc()
    return _cache["nc"]


def run_sharded(x, trace=False, **kw):
    from concourse.bass_utils import run_bass_kernel_spmd

    nc = _get_nc()
    x = np.ascontiguousarray(np.asarray(x), dtype=np.float32)
    in_maps = [{"x": x[i * B : (i + 1) * B]} for i in range(N_CORES)]
    res = run_bass_kernel_spmd(
        nc, in_maps, core_ids=list(range(N_CORES)), trace=trace, **kw
    )
    full = np.concatenate([res.results[i]["out"] for i in range(N_CORES)], axis=0)
    return full, res


def kernel(x):
    full, _ = run_sharded(x)
    return full
